# revision 76
# baseline (speedup 1.0000x reference)
"""Causal self-attention (B=2, T=2048, C=1024, H=16) on 8 Trainium2 NeuronCores.

Sharding (Megatron-style): core c handles batch b = c//4 and head group
g = c%4 (4 heads each).  c_attn is column-parallel, c_proj row-parallel with
the 4 partial outputs per batch summed on the host (+ b_proj).

Single dense PE stream per core (251us baseline -> ~169us):
  - QKV projection groups, attention S=K^T Q + exp + YT=V_aug^T P units, and
    the output projection are interleaved into one instruction stream.
    Attention units (ho, ib) run ib-outer so each 512-query i-block is
    normalized and projected as soon as both head pairs finish, instead of in
    a serial tail.  While a unit's S->exp->YT pipeline runs (ScalarE exp is
    the per-slot rate limiter at ~1150ns vs the PE's ~850ns), independent
    QKV / proj matmul chunks are pulled from a filler queue, one per j-block
    slot, so the PE never idles and its clock stays ramped (matmuls run
    ~1.5x slower until ~3us of continuous execution).
  - K^T is stored zero-padded per head pair (ktz): the PE runs 64-row
    contractions ~1.5x slower than 128-row ones, so each head's [64,128]
    stationary is embedded in a [128,128] tile whose other half is zero and
    the moving qt supplies both heads' rows.
  - The causal mask is a second matmul in the S accumulation group
    (stp += I^T @ tri_bf16), PE-internal, keeping DVE/ScalarE hops off the
    S->exp critical path.
  - Softmax denominators (ones column of V_aug -> YT row 64) are normalized
    per i-block: half-lane reciprocal_approx_fast (rows on partitions 0/32),
    partition-broadcast via a [128,128] selection matmul into PSUM, then one
    fused multiply+bf16-cast into yt2.  Each unit's normalize chain is
    emitted AFTER the next unit's first S pairs: its broadcast matmul is
    gated by the DVE reciprocal and would otherwise head-of-line block
    independent S matmuls in the PE FIFO.
  - DMAs: host pre-arranges all inputs so every transfer has 4KB+ contiguous
    partition lines; few large DMAs beat many small ones (per-DMA trigger
    cost dominates queue parallelism).  Output is bf16 (host upcasts and
    sums the row-parallel partials in f32).
"""

import os
import sys
import types
from contextlib import ExitStack

import ml_dtypes
import numpy as np

for _p in ("/opt/trn_rl_repo",):
    if os.path.isdir(_p) and _p not in sys.path:
        sys.path.append(_p)
os.environ.setdefault("JAX_PLATFORMS", "cpu")

import concourse.bass as bass
import concourse.tile as tile
from concourse import bacc, mybir
from concourse.bass_utils import run_bass_kernel_spmd

B, T, C, H = 2, 2048, 1024, 16
P = 128
CO = C // P          # 8 contraction blocks for the qkv projection
HL = H // 4          # 4 local heads per core
D = C // H           # 64
NB = T // 512        # 4 i-blocks of 512
NEG = -1.0e30
F32 = mybir.dt.float32
BF16 = mybir.dt.bfloat16
EXPF = mybir.ActivationFunctionType.Exp
ADD = mybir.AluOpType.add
MULT = mybir.AluOpType.mult

_CACHE = {}


def _install_ntff_hook():
    """Agent image's antenv lacks axon_hooks; recreate so trace=True works."""
    try:
        from antenv import axon_hooks  # noqa: F401
        return
    except ImportError:
        pass
    try:
        import antenv
        from trn_agent_boot.trn_boot import _ntff_profile_via_ctypes
    except ImportError:
        return
    mod = types.ModuleType("antenv.axon_hooks")
    _hook = [None]
    mod.set_axon_ntff_profile_hook = lambda h: _hook.__setitem__(0, h)
    mod.get_axon_ntff_profile_hook = lambda: _hook[0]
    sys.modules["antenv.axon_hooks"] = mod
    antenv.axon_hooks = mod
    so = "/opt/axon/libaxon_pjrt.so"
    if os.path.exists(so):
        mod.set_axon_ntff_profile_hook(_ntff_profile_via_ctypes(so))


def build_module(dbg=False):
    nc = bacc.Bacc("TRN2", target_bir_lowering=False, debug=False, num_devices=8)
    dbg_d = {}
    if dbg:
        dbg_d["qt"] = nc.dram_tensor("qt_dbg", [P, 2, T], BF16,
                                     kind="ExternalOutput").ap()
        dbg_d["kt"] = nc.dram_tensor("kt_dbg", [P, 2, T], BF16,
                                     kind="ExternalOutput").ap()
        dbg_d["yt2"] = nc.dram_tensor("yt2_dbg", [P, 2, T], BF16,
                                      kind="ExternalOutput").ap()
        dbg_d["den"] = nc.dram_tensor("den_dbg", [8, 2, 512], F32,
                                      kind="ExternalOutput").ap()
        dbg_d["rden"] = nc.dram_tensor("rden_dbg", [8, 2, 512], F32,
                                       kind="ExternalOutput").ap()
        dbg_d["rdb"] = nc.dram_tensor("rdb_dbg", [8, P, 512], F32,
                                      kind="ExternalOutput").ap()

    # host pre-arranged layouts: every DMA partition line is >=4KB contiguous
    xt_d = nc.dram_tensor("xt", [NB, P, CO * 512], BF16,
                          kind="ExternalInput").ap()
    wq_d = nc.dram_tensor("wq", [P, CO * 256], BF16, kind="ExternalInput").ap()
    wk_d = nc.dram_tensor("wk", [P, CO * 256], BF16, kind="ExternalInput").ap()
    wv_d = nc.dram_tensor("wv", [P, CO * 256], BF16, kind="ExternalInput").ap()
    wp_d = nc.dram_tensor("wp", [P, 2 * C], BF16, kind="ExternalInput").ap()
    sel_d = nc.dram_tensor("sel", [P, P], BF16, kind="ExternalInput").ap()
    bq_d = nc.dram_tensor("bq", [256], F32, kind="ExternalInput").ap()
    bk_d = nc.dram_tensor("bk", [256], F32, kind="ExternalInput").ap()
    bv_d = nc.dram_tensor("bv", [256], F32, kind="ExternalInput").ap()
    tri_d = nc.dram_tensor("tri", [P, P], BF16, kind="ExternalInput").ap()
    idm_d = nc.dram_tensor("idm", [P, P], BF16, kind="ExternalInput").ap()
    out_d = nc.dram_tensor("out", [T, C], BF16, kind="ExternalOutput").ap()

    with tile.TileContext(nc) as tc, ExitStack() as ctx:
        const = ctx.enter_context(tc.tile_pool(name="const", bufs=1))
        s1w = ctx.enter_context(tc.tile_pool(name="s1w", bufs=1))
        # PSUM: 8 banks of [128, 512]f32 total.  acc(2) + stp(4) + ytp(2).
        psA = ctx.enter_context(tc.tile_pool(name="psA", bufs=2, space="PSUM"))
        psS = ctx.enter_context(tc.tile_pool(name="psS", bufs=4, space="PSUM"))
        psY = ctx.enter_context(tc.tile_pool(name="psY", bufs=2, space="PSUM"))
        ppool = ctx.enter_context(tc.tile_pool(name="ppool", bufs=8))
        dpool = ctx.enter_context(tc.tile_pool(name="dpool", bufs=3))
        bpool = ctx.enter_context(tc.tile_pool(name="bpool", bufs=3))
        opool = ctx.enter_context(tc.tile_pool(name="opool", bufs=4))

        # ---- persistent SBUF tensors -------------------------------------
        qt = const.tile([P, 2, T], BF16, tag="qt")     # [d, do, t]; head pair per do
        # K^T zero-padded per head: z=0 keeps rows 0-63 (hp0), z=1 rows 64-127
        # (hp1), the other half zeroed -> S matmuls contract over K=128 (the
        # PE runs 64-contraction matmuls ~1.5x slower than 128)
        ktz = const.tile([P, 2, 2, T], BF16, tag="ktz")  # [d, z, do, t]
        vsb = const.tile([P, T // P, HL, 66], BF16, tag="vsb")  # [tp, to, l, 1|V|1]
        yt2 = const.tile([P, 2, T], BF16, tag="yt2")   # Y^T (normalized)
        wp_sb = const.tile([P, 2, C], BF16, tag="wp")
        tri_sb = const.tile([P, P], BF16, tag="tri")
        idm_sb = const.tile([P, P], BF16, tag="idm")
        bq_sb = const.tile([P, 2], F32, tag="bq")
        bk_sb = const.tile([P, 2], F32, tag="bk")
        bv_sb = const.tile([P, 256], F32, tag="bv")

        xt_sb = s1w.tile([P, NB, CO, 512], BF16, tag="xt")  # t4-major
        wq_sb = s1w.tile([P, 2, CO, P], BF16, tag="wq")   # do-major halves
        wk_sb = s1w.tile([P, 2, CO, P], BF16, tag="wk")
        wv_sb = s1w.tile([P, CO, 256], BF16, tag="wv")
        sel_sb = const.tile([P, P], BF16, tag="sel")
        rbpair = const.tile([P, 512], BF16, tag="rbpair")

        # ---- input DMA, priority order -----------------------------------
        # tiny constants first (first diag S needs tri; bias adds need b*)
        nc.sync.dma_start(tri_sb[:], tri_d)
        nc.sync.dma_start(idm_sb[:], idm_d)
        nc.sync.dma_start(bq_sb[:], bq_d.rearrange("(do p) -> p do", p=P))
        nc.sync.dma_start(bk_sb[:], bk_d.rearrange("(do p) -> p do", p=P))
        nc.sync.dma_start(
            bv_sb[:],
            bass.AP(tensor=bv_d.tensor, offset=bv_d.offset,
                    ap=[[0, P]] + list(bv_d.ap)),
        )
        nc.sync.dma_start(sel_sb[:], sel_d)

        # first-needed halves first so the prologue unblocks sooner
        nc.sync.dma_start(wq_sb[:, 0], wq_d[:, 0:CO * P])
        nc.sync.dma_start(wk_sb[:, 0], wk_d[:, 0:CO * P])
        nc.sync.dma_start(xt_sb[:, 0, 0:4], xt_d[0][:, 0:4 * 512])
        nc.sync.dma_start(xt_sb[:, 0, 4:8], xt_d[0][:, 4 * 512:])
        nc.sync.dma_start(wv_sb[:], wv_d)
        nc.sync.dma_start(wq_sb[:, 1], wq_d[:, CO * P:])
        nc.sync.dma_start(wk_sb[:, 1], wk_d[:, CO * P:])
        nc.sync.dma_start(xt_sb[:, 1, 0:4], xt_d[1][:, 0:4 * 512])
        nc.sync.dma_start(xt_sb[:, 1, 4:8], xt_d[1][:, 4 * 512:])
        for t4 in range(2, NB):
            nc.sync.dma_start(xt_sb[:, t4], xt_d[t4])
        nc.sync.dma_start(wp_sb[:], wp_d)
        nc.vector.memset(vsb[:, :, :, 65:66], 1.0)
        nc.vector.memset(rbpair[:], 0.0)
        nc.gpsimd.memset(ktz[0:64, 1], 0.0)
        nc.gpsimd.memset(ktz[64:P, 0], 0.0)

        # ---- matmul group emitters ---------------------------------------
        def qk_chunks(w_sb, b_sb, dst, do, t4):
            """QT/KT d-major: psum[d, t] = W[:, dcols]^T @ x^T; 4 chunks.
            dst=None writes the zero-padded ktz halves instead."""
            st = {}
            tw = slice(t4 * 512, (t4 + 1) * 512)

            def chunk(c0, w):
                def go():
                    if c0 == 0:
                        st["ps"] = psA.tile([P, 512], F32, tag="acc", name="qkps")
                    ps = st["ps"]
                    for co in range(c0, c0 + w):
                        nc.tensor.matmul(
                            ps[:],
                            lhsT=w_sb[:, do, co, :],
                            rhs=xt_sb[:, t4, co, :],
                            start=(co == 0), stop=(co == CO - 1),
                        )
                    if c0 + w == CO:
                        if dst is None:
                            nc.vector.tensor_scalar_add(
                                ktz[0:64, 0, do, tw], ps[0:64],
                                b_sb[0:64, do:do + 1])
                            nc.vector.tensor_scalar_add(
                                ktz[64:P, 1, do, tw], ps[64:P],
                                b_sb[64:P, do:do + 1])
                        else:
                            nc.vector.tensor_scalar_add(
                                dst[:, do, tw], ps[:], b_sb[:, do:do + 1])
                return go
            return [chunk(0, 3), chunk(3, 3), chunk(6, 2)]

        def v_chunks(to):
            """V t-major: psum[t, d] = x^T-block^T @ Wv; 2 chunks."""
            st = {}

            def chunk(c0):
                def go():
                    if c0 == 0:
                        st["ps"] = psA.tile(
                            [P, 512], F32, tag="acc", name="vps")[:, 0:256]
                    ps = st["ps"]
                    for co in range(c0, c0 + 4):
                        nc.tensor.matmul(
                            ps[:],
                            lhsT=xt_sb[:, to // 4, co,
                                       (to % 4) * P:(to % 4 + 1) * P],
                            rhs=wv_sb[:, co, :],
                            start=(co == 0), stop=(co == CO - 1),
                        )
                    if c0 == CO - 4:
                        nc.vector.tensor_tensor(
                            vsb[:, to, :, 1:65],
                            ps.rearrange("p (l e) -> p l e", l=HL),
                            bv_sb[:].rearrange("p (l e) -> p l e", l=HL),
                            op=ADD,
                        )
                return go
            return [chunk(c0) for c0 in range(0, CO, 4)]

        def proj_chunks(i1):
            """out[i, :] partial = sum_ho YT^T @ W_proj; 2 chunks (nh halves).
            Full-C [128, 1024] staging tile -> 4KB DMA lines, 4 queues."""
            isl = slice(i1 * P, (i1 + 1) * P)
            st = {}

            def chunk(nh):
                nsl = slice(nh * 512, (nh + 1) * 512)

                def go():
                    if nh == 0:
                        st["ot"] = opool.tile([P, C], BF16, tag="ot",
                                              name="ot")
                    ps = psA.tile([P, 512], F32, tag="acc", name="pps")
                    for ho in range(2):
                        nc.tensor.matmul(
                            ps[:], lhsT=yt2[:, ho, isl], rhs=wp_sb[:, ho, nsl],
                            start=(ho == 0), stop=(ho == 1))
                    nc.vector.tensor_copy(st["ot"][:, nsl], ps[:])
                    if nh == 1:
                        if i1 >= 12:  # tail: split across 2 queues
                            nc.sync.dma_start(out_d[isl, :][0:64], st["ot"][0:64])
                            nc.sync.dma_start(out_d[isl, :][64:P], st["ot"][64:P])
                        else:
                            nc.sync.dma_start(out_d[isl, :], st["ot"][:])
                return go
            return [chunk(0), chunk(1)]

        # ---- filler queue: chunks pulled into attention-unit slots -------
        fillers = []
        pos = [0, 0]  # group idx, chunk idx

        def pull_chunk():
            gi, ci = pos
            while gi < len(fillers) and ci >= len(fillers[gi]):
                gi, ci = gi + 1, 0
            if gi < len(fillers):
                fillers[gi][ci]()
                ci += 1
            pos[0], pos[1] = gi, ci

        def drain_through(gidx):
            while True:
                gi, ci = pos
                while gi < len(fillers) and ci >= len(fillers[gi]):
                    gi, ci = gi + 1, 0
                pos[0], pos[1] = gi, ci
                if gi >= gidx or gi >= len(fillers):
                    return
                pull_chunk()

        # do=1 t4=0 groups are fillers before unit (1, 0)
        fillers.append(qk_chunks(wq_sb, bq_sb, qt, 1, 0))
        fillers.append(qk_chunks(wk_sb, bk_sb, None, 1, 0))
        for t4 in range(1, NB):
            fillers.append(qk_chunks(wq_sb, bq_sb, qt, 0, t4))
            fillers.append(qk_chunks(wk_sb, bk_sb, None, 0, t4))
            fillers.append(qk_chunks(wq_sb, bq_sb, qt, 1, t4))
            fillers.append(qk_chunks(wk_sb, bk_sb, None, 1, t4))
            for to in range(4 * t4, 4 * t4 + 4):
                fillers.append(v_chunks(to))
        # watermarks: groups that must be fully drained before unit (ho, ib)
        wm = {(0, 0): 0, (1, 0): 2, (0, 1): 2 + 8, (1, 1): 2 + 8,
              (0, 2): 2 + 16, (1, 2): 2 + 16, (0, 3): 2 + 24, (1, 3): 2 + 24}

        # ---- PE warm-up: dummy matmuls on the tiny const tiles ramp the
        # tensor-engine clock (full speed needs ~3us of continuous work)
        # while the big input DMAs stream in; they absorb head idle time.
        def warmup(n):
            for _ in range(n):
                wps = psS.tile([P, 512], F32, tag="stp", name="wps")
                nc.tensor.matmul(wps[:, 0:P], lhsT=idm_sb[:], rhs=tri_sb[:],
                                 start=True, stop=True)

        # ---- PE warm-up: dummy matmuls on the tiny const tiles ramp the
        # tensor-engine clock while the big input DMAs stream in
        for _ in range(24):
            wps = psA.tile([P, 512], F32, tag="acc", name="wps")
            nc.tensor.matmul(wps[:, 0:P], lhsT=idm_sb[:], rhs=tri_sb[:],
                             start=True, stop=True)

        # ---- prologue: QKV for (do=0, t4=0) + V(to=0..3), emitted direct.
        # q/k chunks interleaved in DMA-arrival order: the co4-7 chunks wait
        # xt0's second half, and emitting them before the k co0-3 chunks
        # would head-of-line block work whose data already landed.
        qch = qk_chunks(wq_sb, bq_sb, qt, 0, 0)
        kch = qk_chunks(wk_sb, bk_sb, None, 0, 0)
        for ch in (qch[0], kch[0], qch[1], kch[1], qch[2], kch[2]):
            ch()
        for to in range(4):
            for ch in v_chunks(to):
                ch()

        # ---- attention unit (ho, ib): S -> exp -> YT, then normalize -----
        def attention_unit(ho, ib, prev_fin=None):
            njb = 4 * ib + 4
            ytp = []

            def win(jb):
                r = jb - 4 * ib
                i0 = jb * P if r >= 0 else ib * 512
                return r, i0, (ib + 1) * 512 - i0

            pts = {}

            def emit_st(jb):
                r, i0, N = win(jb)
                jsl = slice(jb * P, (jb + 1) * P)
                pair = []
                for hp in range(2):
                    pb = hp * 64
                    stp = psS.tile([P, 512], F32, tag="stp")
                    nc.tensor.matmul(
                        stp[:, :N], lhsT=ktz[:, hp, ho, jsl],
                        rhs=qt[:, ho, i0:i0 + N],
                        start=True, stop=(r < 0))
                    if r >= 0:
                        # causal mask folded into the accumulation group:
                        # stp[:, :P] += I^T @ tri  (PE-internal, no DVE hop)
                        nc.tensor.matmul(
                            stp[:, 0:P], lhsT=idm_sb[:], rhs=tri_sb[:],
                            start=False, stop=True)
                    pt = ppool.tile([P, 512], BF16, tag="pt")
                    nc.scalar.activation(pt[:, :N], stp[:, :N], EXPF,
                                         scale=float(1.0 / np.sqrt(D)))
                    pair.append(pt)
                pts[jb] = pair

            def emit_yt(jb):
                _, i0, N = win(jb)
                f0 = i0 - ib * 512
                last = jb == njb - 1
                pair = pts.pop(jb)
                for hp in range(2):
                    nc.tensor.matmul(
                        ytp[hp][0:65, f0:f0 + N],
                        lhsT=vsb[:, jb, 2 * ho + hp, 1:66],
                        rhs=pair[hp][:, :N], start=(jb == 0), stop=last)

            def finalize():
                normalize_unit(ho, ib, ytp)

            # software pipeline, two ST-pairs ahead of the YT accumulation.
            # The previous unit's normalize chain is emitted AFTER this unit's
            # first S pairs: its broadcast matmul is gated by the DVE
            # reciprocal, and emitting it last would head-of-line block these
            # independent S matmuls in the PE FIFO.
            emit_st(0)
            if njb > 1:
                emit_st(1)
            if prev_fin is not None:
                prev_fin()
            if ib == 3:
                # cover the previous unit's normalize latency (ytp-bank WAR)
                # with extra fillers; ration the rest across the long unit
                for _ in range(5):
                    pull_chunk()
            ytp.extend(psY.tile([P, 512], F32, tag="ytp", name=f"ytp_{hp}")
                       for hp in range(2))
            for jb in range(njb):
                if jb + 2 < njb:
                    emit_st(jb + 2)
                if not (ib == 3 and jb % 2 == 1):
                    pull_chunk()
                emit_yt(jb)
            return finalize

        def normalize_unit(ho, ib, ytp):
            # normalize this i-block in place: stack denominators, fast
            # reciprocal, PE broadcast matmul, fused mult+bf16 cast
            iw = slice(ib * 512, (ib + 1) * 512)
            # den rows on partitions 0/32 (32-aligned) -> half-lane reciprocal
            den2 = dpool.tile([33, 512], F32, tag="den2")
            for hp in range(2):
                nc.vector.tensor_copy(
                    den2[32 * hp:32 * hp + 1, :], ytp[hp][64:65, :])
            rden2 = dpool.tile([33, 512], F32, tag="rden2")
            nc.vector.reciprocal_approx_fast(out=rden2[:], in_=den2[:])
            # broadcast 1/den across partitions with a tiny f32r matmul:
            # rb[m, i] = sum_k sel[k, m] * rbpair[k, i]; sel rows 0/32 select
            # the hp0/hp1 reciprocal rows into partitions 0-63 / 64-127.
            nc.vector.tensor_copy(rbpair[0:1, :], rden2[0:1, :])
            nc.vector.tensor_copy(rbpair[32:33, :], rden2[32:33, :])
            rb = psA.tile([P, 512], F32, tag="acc", name="rb")
            nc.tensor.matmul(
                rb[:], lhsT=sel_sb[:], rhs=rbpair[:], start=True, stop=True)
            rbs = bpool.tile([P, 512], F32, tag="rbs")
            nc.vector.tensor_copy(rbs[:], rb[:])
            for hp in range(2):
                pb = hp * 64
                nc.vector.tensor_tensor(
                    yt2[pb:pb + 64, ho, iw], ytp[hp][0:64, :],
                    rbs[pb:pb + 64, :], op=MULT)
            if dbg:
                u = ho * NB + ib
                nc.sync.dma_start(dbg_d["den"][u, 0], den2[0])
                nc.sync.dma_start(dbg_d["den"][u, 1], den2[32])
                nc.sync.dma_start(dbg_d["rden"][u, 0], rden2[0])
                nc.sync.dma_start(dbg_d["rden"][u, 1], rden2[32])

        # ---- main interleaved stream -------------------------------------
        fin = None
        for ib in range(NB):
            for ho in range(2):
                drain_through(wm[(ho, ib)])
                fin = attention_unit(ho, ib, prev_fin=fin)
            for i1 in range(4 * ib, 4 * ib + 4):
                fillers.append(proj_chunks(i1))
        fin()
        drain_through(len(fillers))
        if dbg:
            nc.sync.dma_start(dbg_d["qt"][:], qt[:])
            nc.sync.dma_start(dbg_d["kt"][0:64], ktz[0:64, 0])
            nc.sync.dma_start(dbg_d["kt"][64:P], ktz[64:P, 1])
            nc.sync.dma_start(dbg_d["yt2"][:], yt2[:])

    nc.compile()
    return nc


def _get_module():
    if "nc" not in _CACHE:
        _CACHE["nc"] = build_module()
    return _CACHE["nc"]


def _make_in_maps(x, W_attn, b_attn, W_proj):
    bf = ml_dtypes.bfloat16
    tri = np.where(np.arange(P)[None, :] >= np.arange(P)[:, None],
                   np.float32(0.0), np.float32(NEG)).astype(bf)
    idm = np.eye(P, dtype=np.float32).astype(bf)
    sel = np.zeros((P, P), np.float32)
    sel[0, 0:64] = 1.0
    sel[32, 64:128] = 1.0
    sel = sel.astype(bf)

    def wlay(w):  # [C, 256] -> [P, do, co, 128] flat (do-major halves)
        return np.ascontiguousarray(
            w.reshape(CO, P, 2, P).transpose(1, 2, 0, 3).reshape(P, CO * 256)
        ).astype(bf)

    def wlay_v(w):  # [C, 256] -> [P, CO*256], co-major (V keeps old layout)
        return np.ascontiguousarray(
            w.reshape(CO, P, 256).transpose(1, 0, 2).reshape(P, CO * 256)
        ).astype(bf)

    in_maps = []
    for core in range(8):
        b, g = divmod(core, 4)
        cs = slice(g * 256, (g + 1) * 256)
        xT = x[b].T  # [C, T]
        xtb = np.ascontiguousarray(
            xT.reshape(CO, P, NB, 512).transpose(2, 1, 0, 3).reshape(
                NB, P, CO * 512)).astype(bf)
        wpb = np.ascontiguousarray(
            W_proj[cs, :].reshape(2, P, C).transpose(1, 0, 2).reshape(
                P, 2 * C)).astype(bf)
        in_maps.append({
            "xt": xtb,
            "wq": wlay(W_attn[:, g * 256:(g + 1) * 256]),
            "wk": wlay(W_attn[:, C + g * 256:C + (g + 1) * 256]),
            "wv": wlay_v(W_attn[:, 2 * C + g * 256:2 * C + (g + 1) * 256]),
            "wp": wpb,
            "bq": np.ascontiguousarray(b_attn[cs]),
            "bk": np.ascontiguousarray(b_attn[C + g * 256:C + (g + 1) * 256]),
            "bv": np.ascontiguousarray(b_attn[2 * C + g * 256:2 * C + (g + 1) * 256]),
            "tri": tri,
            "idm": idm,
            "sel": sel,
        })
    return in_maps


def _gather(results, b_proj):
    y = np.empty((B, T, C), np.float32)
    for b in range(B):
        acc = results[4 * b]["out"].astype(np.float32).copy()
        for g in range(1, 4):
            acc += results[4 * b + g]["out"]
        y[b] = acc + b_proj[None, :].astype(np.float32)
    return y


def kernel(x, W_attn, b_attn, W_proj, b_proj, _trace=False):
    x = np.asarray(x, np.float32)
    W_attn = np.asarray(W_attn, np.float32)
    b_attn = np.asarray(b_attn, np.float32)
    W_proj = np.asarray(W_proj, np.float32)
    b_proj = np.asarray(b_proj, np.float32)

    nc = _get_module()
    in_maps = _make_in_maps(x, W_attn, b_attn, W_proj)
    kw = {}
    if _trace:
        _install_ntff_hook()
        kw = dict(trace=True)
    res = run_bass_kernel_spmd(nc, in_maps, core_ids=list(range(8)), **kw)
    out = _gather(res.results, b_proj)
    if _trace:
        return out, res
    return out


# revision 77
# speedup vs baseline: 1.0222x; 1.0222x over previous
"""Causal self-attention (B=2, T=2048, C=1024, H=16) on 8 Trainium2 NeuronCores.

Sharding (Megatron-style): core c handles batch b = c//4 and head group
g = c%4 (4 heads each).  c_attn is column-parallel, c_proj row-parallel with
the 4 partial outputs per batch summed on the host (+ b_proj).

Single dense PE stream per core (251us baseline -> ~169us):
  - QKV projection groups, attention S=K^T Q + exp + YT=V_aug^T P units, and
    the output projection are interleaved into one instruction stream.
    Attention units (ho, ib) run ib-outer so each 512-query i-block is
    normalized and projected as soon as both head pairs finish, instead of in
    a serial tail.  While a unit's S->exp->YT pipeline runs (ScalarE exp is
    the per-slot rate limiter at ~1150ns vs the PE's ~850ns), independent
    QKV / proj matmul chunks are pulled from a filler queue, one per j-block
    slot, so the PE never idles and its clock stays ramped (matmuls run
    ~1.5x slower until ~3us of continuous execution).
  - K^T is stored zero-padded per head pair (ktz): the PE runs 64-row
    contractions ~1.5x slower than 128-row ones, so each head's [64,128]
    stationary is embedded in a [128,128] tile whose other half is zero and
    the moving qt supplies both heads' rows.
  - The causal mask is a second matmul in the S accumulation group
    (stp += I^T @ tri_bf16), PE-internal, keeping DVE/ScalarE hops off the
    S->exp critical path.
  - Softmax denominators (ones column of V_aug -> YT row 64) are normalized
    per i-block: half-lane reciprocal_approx_fast (rows on partitions 0/32),
    partition-broadcast via a [128,128] selection matmul into PSUM, then one
    fused multiply+bf16-cast into yt2.  Each unit's normalize chain is
    emitted AFTER the next unit's first S pairs: its broadcast matmul is
    gated by the DVE reciprocal and would otherwise head-of-line block
    independent S matmuls in the PE FIFO.
  - DMAs: host pre-arranges all inputs so every transfer has 4KB+ contiguous
    partition lines; few large DMAs beat many small ones (per-DMA trigger
    cost dominates queue parallelism).  Output is bf16 (host upcasts and
    sums the row-parallel partials in f32).
"""

import os
import sys
import types
from contextlib import ExitStack

import ml_dtypes
import numpy as np

for _p in ("/opt/trn_rl_repo",):
    if os.path.isdir(_p) and _p not in sys.path:
        sys.path.append(_p)
os.environ.setdefault("JAX_PLATFORMS", "cpu")

import concourse.bass as bass
import concourse.tile as tile
from concourse import bacc, mybir
from concourse.bass_utils import run_bass_kernel_spmd

B, T, C, H = 2, 2048, 1024, 16
P = 128
CO = C // P          # 8 contraction blocks for the qkv projection
HL = H // 4          # 4 local heads per core
D = C // H           # 64
NB = T // 512        # 4 i-blocks of 512
NEG = -1.0e30
F32 = mybir.dt.float32
BF16 = mybir.dt.bfloat16
EXPF = mybir.ActivationFunctionType.Exp
ADD = mybir.AluOpType.add
MULT = mybir.AluOpType.mult

_CACHE = {}


def _install_ntff_hook():
    """Agent image's antenv lacks axon_hooks; recreate so trace=True works."""
    try:
        from antenv import axon_hooks  # noqa: F401
        return
    except ImportError:
        pass
    try:
        import antenv
        from trn_agent_boot.trn_boot import _ntff_profile_via_ctypes
    except ImportError:
        return
    mod = types.ModuleType("antenv.axon_hooks")
    _hook = [None]
    mod.set_axon_ntff_profile_hook = lambda h: _hook.__setitem__(0, h)
    mod.get_axon_ntff_profile_hook = lambda: _hook[0]
    sys.modules["antenv.axon_hooks"] = mod
    antenv.axon_hooks = mod
    so = "/opt/axon/libaxon_pjrt.so"
    if os.path.exists(so):
        mod.set_axon_ntff_profile_hook(_ntff_profile_via_ctypes(so))


def build_module(dbg=False):
    nc = bacc.Bacc("TRN2", target_bir_lowering=False, debug=False, num_devices=8)
    dbg_d = {}
    if dbg:
        dbg_d["qt"] = nc.dram_tensor("qt_dbg", [P, 2, T], BF16,
                                     kind="ExternalOutput").ap()
        dbg_d["kt"] = nc.dram_tensor("kt_dbg", [P, 2, T], BF16,
                                     kind="ExternalOutput").ap()
        dbg_d["yt2"] = nc.dram_tensor("yt2_dbg", [P, 2, T], BF16,
                                      kind="ExternalOutput").ap()
        dbg_d["den"] = nc.dram_tensor("den_dbg", [8, 2, 512], F32,
                                      kind="ExternalOutput").ap()
        dbg_d["rden"] = nc.dram_tensor("rden_dbg", [8, 2, 512], F32,
                                       kind="ExternalOutput").ap()
        dbg_d["rdb"] = nc.dram_tensor("rdb_dbg", [8, P, 512], F32,
                                      kind="ExternalOutput").ap()

    # host pre-arranged layouts: every DMA partition line is >=4KB contiguous
    xt_d = nc.dram_tensor("xt", [NB, P, CO * 512], BF16,
                          kind="ExternalInput").ap()
    wq_d = nc.dram_tensor("wq", [P, CO * 256], BF16, kind="ExternalInput").ap()
    wk_d = nc.dram_tensor("wk", [P, CO * 256], BF16, kind="ExternalInput").ap()
    wv_d = nc.dram_tensor("wv", [P, CO * 256], BF16, kind="ExternalInput").ap()
    wp_d = nc.dram_tensor("wp", [P, 2 * C], BF16, kind="ExternalInput").ap()
    sel_d = nc.dram_tensor("sel", [P, P], BF16, kind="ExternalInput").ap()
    bq_d = nc.dram_tensor("bq", [256], F32, kind="ExternalInput").ap()
    bk_d = nc.dram_tensor("bk", [256], F32, kind="ExternalInput").ap()
    bv_d = nc.dram_tensor("bv", [256], F32, kind="ExternalInput").ap()
    tri_d = nc.dram_tensor("tri", [P, P], BF16, kind="ExternalInput").ap()
    idm_d = nc.dram_tensor("idm", [P, P], BF16, kind="ExternalInput").ap()
    out_d = nc.dram_tensor("out", [T, C], BF16, kind="ExternalOutput").ap()

    with tile.TileContext(nc) as tc, ExitStack() as ctx:
        const = ctx.enter_context(tc.tile_pool(name="const", bufs=1))
        s1w = ctx.enter_context(tc.tile_pool(name="s1w", bufs=1))
        # PSUM: 8 banks of [128, 512]f32 total.  acc(2) + stp(4) + ytp(2).
        psA = ctx.enter_context(tc.tile_pool(name="psA", bufs=2, space="PSUM"))
        psS = ctx.enter_context(tc.tile_pool(name="psS", bufs=4, space="PSUM"))
        psY = ctx.enter_context(tc.tile_pool(name="psY", bufs=2, space="PSUM"))
        ppool = ctx.enter_context(tc.tile_pool(name="ppool", bufs=8))
        dpool = ctx.enter_context(tc.tile_pool(name="dpool", bufs=3))
        bpool = ctx.enter_context(tc.tile_pool(name="bpool", bufs=3))
        opool = ctx.enter_context(tc.tile_pool(name="opool", bufs=4))

        # ---- persistent SBUF tensors -------------------------------------
        qt = const.tile([P, 2, T], BF16, tag="qt")     # [d, do, t]; head pair per do
        # K^T zero-padded per head: z=0 keeps rows 0-63 (hp0), z=1 rows 64-127
        # (hp1), the other half zeroed -> S matmuls contract over K=128 (the
        # PE runs 64-contraction matmuls ~1.5x slower than 128)
        ktz = const.tile([P, 2, 2, T], BF16, tag="ktz")  # [d, z, do, t]
        vsb = const.tile([P, T // P, HL, 66], BF16, tag="vsb")  # [tp, to, l, 1|V|1]
        yt2 = const.tile([P, 2, T], BF16, tag="yt2")   # Y^T (normalized)
        wp_sb = const.tile([P, 2, C], BF16, tag="wp")
        tri_sb = const.tile([P, P], BF16, tag="tri")
        idm_sb = const.tile([P, P], BF16, tag="idm")
        bq_sb = const.tile([P, 2], F32, tag="bq")
        bk_sb = const.tile([P, 2], F32, tag="bk")
        bv_sb = const.tile([P, 256], F32, tag="bv")

        xt_sb = s1w.tile([P, NB, CO, 512], BF16, tag="xt")  # t4-major
        wq_sb = s1w.tile([P, 2, CO, P], BF16, tag="wq")   # do-major halves
        wk_sb = s1w.tile([P, 2, CO, P], BF16, tag="wk")
        wv_sb = s1w.tile([P, CO, 256], BF16, tag="wv")
        sel_sb = const.tile([P, P], BF16, tag="sel")
        rbpair = const.tile([P, 512], BF16, tag="rbpair")

        # ---- input DMA, priority order -----------------------------------
        # tiny constants first (first diag S needs tri; bias adds need b*)
        nc.sync.dma_start(tri_sb[:], tri_d)
        nc.sync.dma_start(idm_sb[:], idm_d)
        nc.sync.dma_start(bq_sb[:], bq_d.rearrange("(do p) -> p do", p=P))
        nc.sync.dma_start(bk_sb[:], bk_d.rearrange("(do p) -> p do", p=P))
        nc.sync.dma_start(
            bv_sb[:],
            bass.AP(tensor=bv_d.tensor, offset=bv_d.offset,
                    ap=[[0, P]] + list(bv_d.ap)),
        )
        nc.sync.dma_start(sel_sb[:], sel_d)

        # first-needed halves first so the prologue unblocks sooner
        nc.sync.dma_start(wq_sb[:, 0], wq_d[:, 0:CO * P])
        nc.sync.dma_start(wk_sb[:, 0], wk_d[:, 0:CO * P])
        nc.sync.dma_start(xt_sb[:, 0, 0:4], xt_d[0][:, 0:4 * 512])
        nc.sync.dma_start(xt_sb[:, 0, 4:8], xt_d[0][:, 4 * 512:])
        nc.sync.dma_start(wv_sb[:], wv_d)
        nc.sync.dma_start(wq_sb[:, 1], wq_d[:, CO * P:])
        nc.sync.dma_start(wk_sb[:, 1], wk_d[:, CO * P:])
        nc.sync.dma_start(xt_sb[:, 1, 0:4], xt_d[1][:, 0:4 * 512])
        nc.sync.dma_start(xt_sb[:, 1, 4:8], xt_d[1][:, 4 * 512:])
        for t4 in range(2, NB):
            nc.sync.dma_start(xt_sb[:, t4], xt_d[t4])
        nc.sync.dma_start(wp_sb[:], wp_d)
        nc.vector.memset(vsb[:, :, :, 65:66], 1.0)
        nc.vector.memset(rbpair[:], 0.0)
        nc.gpsimd.memset(ktz[0:64, 1], 0.0)
        nc.gpsimd.memset(ktz[64:P, 0], 0.0)

        # ---- matmul group emitters ---------------------------------------
        def qk_chunks(w_sb, b_sb, dst, do, t4):
            """QT/KT d-major: psum[d, t] = W[:, dcols]^T @ x^T; 4 chunks.
            dst=None writes the zero-padded ktz halves instead."""
            st = {}
            tw = slice(t4 * 512, (t4 + 1) * 512)

            def chunk(c0):
                def go():
                    if c0 == 0:
                        st["ps"] = psA.tile([P, 512], F32, tag="acc", name="qkps")
                    ps = st["ps"]
                    for co in range(c0, c0 + 2):
                        nc.tensor.matmul(
                            ps[:],
                            lhsT=w_sb[:, do, co, :],
                            rhs=xt_sb[:, t4, co, :],
                            start=(co == 0), stop=(co == CO - 1),
                        )
                    if c0 == CO - 2:
                        if dst is None:
                            nc.vector.tensor_scalar_add(
                                ktz[0:64, 0, do, tw], ps[0:64],
                                b_sb[0:64, do:do + 1])
                            nc.vector.tensor_scalar_add(
                                ktz[64:P, 1, do, tw], ps[64:P],
                                b_sb[64:P, do:do + 1])
                        else:
                            nc.vector.tensor_scalar_add(
                                dst[:, do, tw], ps[:], b_sb[:, do:do + 1])
                return go
            return [chunk(c0) for c0 in range(0, CO, 2)]

        def v_chunks(to):
            """V t-major: psum[t, d] = x^T-block^T @ Wv; 2 chunks."""
            st = {}

            def chunk(c0):
                def go():
                    if c0 == 0:
                        st["ps"] = psA.tile(
                            [P, 512], F32, tag="acc", name="vps")[:, 0:256]
                    ps = st["ps"]
                    for co in range(c0, c0 + 4):
                        nc.tensor.matmul(
                            ps[:],
                            lhsT=xt_sb[:, to // 4, co,
                                       (to % 4) * P:(to % 4 + 1) * P],
                            rhs=wv_sb[:, co, :],
                            start=(co == 0), stop=(co == CO - 1),
                        )
                    if c0 == CO - 4:
                        nc.vector.tensor_tensor(
                            vsb[:, to, :, 1:65],
                            ps.rearrange("p (l e) -> p l e", l=HL),
                            bv_sb[:].rearrange("p (l e) -> p l e", l=HL),
                            op=ADD,
                        )
                return go
            return [chunk(c0) for c0 in range(0, CO, 4)]

        def proj_chunks(i1):
            """out[i, :] partial = sum_ho YT^T @ W_proj; 2 chunks (nh halves).
            Full-C [128, 1024] staging tile -> 4KB DMA lines, 4 queues."""
            isl = slice(i1 * P, (i1 + 1) * P)
            st = {}

            def chunk(nh):
                nsl = slice(nh * 512, (nh + 1) * 512)

                def go():
                    if nh == 0:
                        st["ot"] = opool.tile([P, C], BF16, tag="ot",
                                              name="ot")
                    ps = psA.tile([P, 512], F32, tag="acc", name="pps")
                    for ho in range(2):
                        nc.tensor.matmul(
                            ps[:], lhsT=yt2[:, ho, isl], rhs=wp_sb[:, ho, nsl],
                            start=(ho == 0), stop=(ho == 1))
                    nc.vector.tensor_copy(st["ot"][:, nsl], ps[:])
                    if nh == 1:
                        if i1 >= 12:  # tail: split across 2 queues
                            nc.sync.dma_start(out_d[isl, :][0:64], st["ot"][0:64])
                            nc.sync.dma_start(out_d[isl, :][64:P], st["ot"][64:P])
                        else:
                            nc.sync.dma_start(out_d[isl, :], st["ot"][:])
                return go
            return [chunk(0), chunk(1)]

        # ---- filler queue: chunks pulled into attention-unit slots -------
        fillers = []
        pos = [0, 0]  # group idx, chunk idx

        def pull_chunk():
            gi, ci = pos
            while gi < len(fillers) and ci >= len(fillers[gi]):
                gi, ci = gi + 1, 0
            if gi < len(fillers):
                fillers[gi][ci]()
                ci += 1
            pos[0], pos[1] = gi, ci

        def drain_through(gidx):
            while True:
                gi, ci = pos
                while gi < len(fillers) and ci >= len(fillers[gi]):
                    gi, ci = gi + 1, 0
                pos[0], pos[1] = gi, ci
                if gi >= gidx or gi >= len(fillers):
                    return
                pull_chunk()

        # do=1 t4=0 groups are fillers before unit (1, 0)
        fillers.append(qk_chunks(wq_sb, bq_sb, qt, 1, 0))
        fillers.append(qk_chunks(wk_sb, bk_sb, None, 1, 0))
        for t4 in range(1, NB):
            fillers.append(qk_chunks(wq_sb, bq_sb, qt, 0, t4))
            fillers.append(qk_chunks(wk_sb, bk_sb, None, 0, t4))
            fillers.append(qk_chunks(wq_sb, bq_sb, qt, 1, t4))
            fillers.append(qk_chunks(wk_sb, bk_sb, None, 1, t4))
            for to in range(4 * t4, 4 * t4 + 4):
                fillers.append(v_chunks(to))
        # watermarks: groups that must be fully drained before unit (ho, ib)
        wm = {(0, 0): 0, (1, 0): 2, (0, 1): 2 + 8, (1, 1): 2 + 8,
              (0, 2): 2 + 16, (1, 2): 2 + 16, (0, 3): 2 + 24, (1, 3): 2 + 24}

        # ---- PE warm-up: dummy matmuls on the tiny const tiles ramp the
        # tensor-engine clock (full speed needs ~3us of continuous work)
        # while the big input DMAs stream in; they absorb head idle time.
        def warmup(n):
            for _ in range(n):
                wps = psS.tile([P, 512], F32, tag="stp", name="wps")
                nc.tensor.matmul(wps[:, 0:P], lhsT=idm_sb[:], rhs=tri_sb[:],
                                 start=True, stop=True)

        # ---- PE warm-up: dummy matmuls on the tiny const tiles ramp the
        # tensor-engine clock while the big input DMAs stream in
        for _ in range(24):
            wps = psA.tile([P, 512], F32, tag="acc", name="wps")
            nc.tensor.matmul(wps[:, 0:P], lhsT=idm_sb[:], rhs=tri_sb[:],
                             start=True, stop=True)

        # ---- prologue: QKV for (do=0, t4=0) + V(to=0..3), emitted direct.
        # q/k chunks interleaved in DMA-arrival order: the co4-7 chunks wait
        # xt0's second half, and emitting them before the k co0-3 chunks
        # would head-of-line block work whose data already landed.
        qch = qk_chunks(wq_sb, bq_sb, qt, 0, 0)
        kch = qk_chunks(wk_sb, bk_sb, None, 0, 0)
        for ch in (qch[0], qch[1], kch[0], kch[1],
                   qch[2], qch[3], kch[2], kch[3]):
            ch()
        for to in range(4):
            for ch in v_chunks(to):
                ch()

        # ---- attention unit (ho, ib): S -> exp -> YT, then normalize -----
        def attention_unit(ho, ib, prev_fin=None):
            njb = 4 * ib + 4
            ytp = []

            def win(jb):
                r = jb - 4 * ib
                i0 = jb * P if r >= 0 else ib * 512
                return r, i0, (ib + 1) * 512 - i0

            pts = {}

            def emit_st(jb):
                r, i0, N = win(jb)
                jsl = slice(jb * P, (jb + 1) * P)
                pair = []
                for hp in range(2):
                    pb = hp * 64
                    stp = psS.tile([P, 512], F32, tag="stp")
                    nc.tensor.matmul(
                        stp[:, :N], lhsT=ktz[:, hp, ho, jsl],
                        rhs=qt[:, ho, i0:i0 + N],
                        start=True, stop=(r < 0))
                    if r >= 0:
                        # causal mask folded into the accumulation group:
                        # stp[:, :P] += I^T @ tri  (PE-internal, no DVE hop)
                        nc.tensor.matmul(
                            stp[:, 0:P], lhsT=idm_sb[:], rhs=tri_sb[:],
                            start=False, stop=True)
                    pt = ppool.tile([P, 512], BF16, tag="pt")
                    nc.scalar.activation(pt[:, :N], stp[:, :N], EXPF,
                                         scale=float(1.0 / np.sqrt(D)))
                    pair.append(pt)
                pts[jb] = pair

            def emit_yt(jb):
                _, i0, N = win(jb)
                f0 = i0 - ib * 512
                last = jb == njb - 1
                pair = pts.pop(jb)
                for hp in range(2):
                    nc.tensor.matmul(
                        ytp[hp][0:65, f0:f0 + N],
                        lhsT=vsb[:, jb, 2 * ho + hp, 1:66],
                        rhs=pair[hp][:, :N], start=(jb == 0), stop=last)

            def finalize():
                normalize_unit(ho, ib, ytp)

            # software pipeline, two ST-pairs ahead of the YT accumulation.
            # The previous unit's normalize chain is emitted AFTER this unit's
            # first S pairs: its broadcast matmul is gated by the DVE
            # reciprocal, and emitting it last would head-of-line block these
            # independent S matmuls in the PE FIFO.
            emit_st(0)
            if njb > 1:
                emit_st(1)
            if prev_fin is not None:
                prev_fin()
            if ib == 3:
                # cover the previous unit's normalize latency (ytp-bank WAR)
                # with extra fillers; ration the rest across the long unit
                for _ in range(5):
                    pull_chunk()
            ytp.extend(psY.tile([P, 512], F32, tag="ytp", name=f"ytp_{hp}")
                       for hp in range(2))
            for jb in range(njb):
                if jb + 2 < njb:
                    emit_st(jb + 2)
                if not (ib == 3 and jb % 2 == 1):
                    pull_chunk()
                emit_yt(jb)
            return finalize

        def normalize_unit(ho, ib, ytp):
            # normalize this i-block in place: stack denominators, fast
            # reciprocal, PE broadcast matmul, fused mult+bf16 cast
            iw = slice(ib * 512, (ib + 1) * 512)
            # den rows on partitions 0/32 (32-aligned) -> half-lane reciprocal
            den2 = dpool.tile([33, 512], F32, tag="den2")
            for hp in range(2):
                nc.vector.tensor_copy(
                    den2[32 * hp:32 * hp + 1, :], ytp[hp][64:65, :])
            rden2 = dpool.tile([33, 512], F32, tag="rden2")
            nc.vector.reciprocal_approx_fast(out=rden2[:], in_=den2[:])
            # broadcast 1/den across partitions with a tiny f32r matmul:
            # rb[m, i] = sum_k sel[k, m] * rbpair[k, i]; sel rows 0/32 select
            # the hp0/hp1 reciprocal rows into partitions 0-63 / 64-127.
            nc.vector.tensor_copy(rbpair[0:1, :], rden2[0:1, :])
            nc.vector.tensor_copy(rbpair[32:33, :], rden2[32:33, :])
            rb = psA.tile([P, 512], F32, tag="acc", name="rb")
            nc.tensor.matmul(
                rb[:], lhsT=sel_sb[:], rhs=rbpair[:], start=True, stop=True)
            rbs = bpool.tile([P, 512], F32, tag="rbs")
            nc.vector.tensor_copy(rbs[:], rb[:])
            for hp in range(2):
                pb = hp * 64
                nc.vector.tensor_tensor(
                    yt2[pb:pb + 64, ho, iw], ytp[hp][0:64, :],
                    rbs[pb:pb + 64, :], op=MULT)
            if dbg:
                u = ho * NB + ib
                nc.sync.dma_start(dbg_d["den"][u, 0], den2[0])
                nc.sync.dma_start(dbg_d["den"][u, 1], den2[32])
                nc.sync.dma_start(dbg_d["rden"][u, 0], rden2[0])
                nc.sync.dma_start(dbg_d["rden"][u, 1], rden2[32])

        # ---- main interleaved stream -------------------------------------
        fin = None
        for ib in range(NB):
            for ho in range(2):
                drain_through(wm[(ho, ib)])
                fin = attention_unit(ho, ib, prev_fin=fin)
            for i1 in range(4 * ib, 4 * ib + 4):
                fillers.append(proj_chunks(i1))
        fin()
        drain_through(len(fillers))
        if dbg:
            nc.sync.dma_start(dbg_d["qt"][:], qt[:])
            nc.sync.dma_start(dbg_d["kt"][0:64], ktz[0:64, 0])
            nc.sync.dma_start(dbg_d["kt"][64:P], ktz[64:P, 1])
            nc.sync.dma_start(dbg_d["yt2"][:], yt2[:])

    nc.compile()
    return nc


def _get_module():
    if "nc" not in _CACHE:
        _CACHE["nc"] = build_module()
    return _CACHE["nc"]


def _make_in_maps(x, W_attn, b_attn, W_proj):
    bf = ml_dtypes.bfloat16
    tri = np.where(np.arange(P)[None, :] >= np.arange(P)[:, None],
                   np.float32(0.0), np.float32(NEG)).astype(bf)
    idm = np.eye(P, dtype=np.float32).astype(bf)
    sel = np.zeros((P, P), np.float32)
    sel[0, 0:64] = 1.0
    sel[32, 64:128] = 1.0
    sel = sel.astype(bf)

    def wlay(w):  # [C, 256] -> [P, do, co, 128] flat (do-major halves)
        return np.ascontiguousarray(
            w.reshape(CO, P, 2, P).transpose(1, 2, 0, 3).reshape(P, CO * 256)
        ).astype(bf)

    def wlay_v(w):  # [C, 256] -> [P, CO*256], co-major (V keeps old layout)
        return np.ascontiguousarray(
            w.reshape(CO, P, 256).transpose(1, 0, 2).reshape(P, CO * 256)
        ).astype(bf)

    in_maps = []
    for core in range(8):
        b, g = divmod(core, 4)
        cs = slice(g * 256, (g + 1) * 256)
        xT = x[b].T  # [C, T]
        xtb = np.ascontiguousarray(
            xT.reshape(CO, P, NB, 512).transpose(2, 1, 0, 3).reshape(
                NB, P, CO * 512)).astype(bf)
        wpb = np.ascontiguousarray(
            W_proj[cs, :].reshape(2, P, C).transpose(1, 0, 2).reshape(
                P, 2 * C)).astype(bf)
        in_maps.append({
            "xt": xtb,
            "wq": wlay(W_attn[:, g * 256:(g + 1) * 256]),
            "wk": wlay(W_attn[:, C + g * 256:C + (g + 1) * 256]),
            "wv": wlay_v(W_attn[:, 2 * C + g * 256:2 * C + (g + 1) * 256]),
            "wp": wpb,
            "bq": np.ascontiguousarray(b_attn[cs]),
            "bk": np.ascontiguousarray(b_attn[C + g * 256:C + (g + 1) * 256]),
            "bv": np.ascontiguousarray(b_attn[2 * C + g * 256:2 * C + (g + 1) * 256]),
            "tri": tri,
            "idm": idm,
            "sel": sel,
        })
    return in_maps


def _gather(results, b_proj):
    y = np.empty((B, T, C), np.float32)
    for b in range(B):
        acc = results[4 * b]["out"].astype(np.float32).copy()
        for g in range(1, 4):
            acc += results[4 * b + g]["out"]
        y[b] = acc + b_proj[None, :].astype(np.float32)
    return y


def kernel(x, W_attn, b_attn, W_proj, b_proj, _trace=False):
    x = np.asarray(x, np.float32)
    W_attn = np.asarray(W_attn, np.float32)
    b_attn = np.asarray(b_attn, np.float32)
    W_proj = np.asarray(W_proj, np.float32)
    b_proj = np.asarray(b_proj, np.float32)

    nc = _get_module()
    in_maps = _make_in_maps(x, W_attn, b_attn, W_proj)
    kw = {}
    if _trace:
        _install_ntff_hook()
        kw = dict(trace=True)
    res = run_bass_kernel_spmd(nc, in_maps, core_ids=list(range(8)), **kw)
    out = _gather(res.results, b_proj)
    if _trace:
        return out, res
    return out
